# revision 6
# baseline (speedup 1.0000x reference)
"""Trainium2 Bass kernel for CrossAttentionInjection (block-diagonal frame attention).

Contract: kernel(**inputs) takes FULL unsharded numpy inputs (as produced by
setup_inputs()) and returns the FULL [B, T, Q_DIM] float32 output.

Sharding: the attention mask is block-diagonal over 8 frames x 256 patches, so
the whole module decomposes into 32 independent (batch, frame) blocks of 256
tokens. Each of the 8 cores processes 4 contiguous blocks (1024 tokens of one
batch) with replicated weights -- no collectives.

Per-core pipeline (bf16 matmuls, fp32 PSUM):
  - DMA-chased startup: xT arrives in token quarters on the sync queue while
    ctx/weights stream on the scalar + gpsimd queues in first-use order; LN
    stats / K-proj / V-proj emissions chase the arrivals so the PE never waits
    on a transfer it does not need yet.
  - LN folded into the Q projection (gamma folded into Wq on the host, mean
    subtracted in-place, 1/std applied in the PSUM->SBUF multiply).
  - Attention is software-pipelined across blocks: scores(b) / AV(b-1) /
    out-proj(b-2) matmuls are interleaved 1:1 big:small so every tiny AV
    matmul's 105ns stationary load hides under the previous matmul's stream.
  - AV PSUM tiles hold 4 heads ([128, 4, 65] with a ones column at 64 for the
    softmax denominator); one reciprocal + one broadcast multiply per tile.
  - O -> O^T via XBAR dma_start_transpose (DMA engine), not PE transposes.
  - Output written bf16 (host upcasts), bias added on gpsimd.
"""

import numpy as np

# ---------------------------------------------------------------------------
# Problem constants (hardcoded; kernel.py must be self-contained)
# ---------------------------------------------------------------------------
B, T, Q_DIM, KV_DIM = 4, 2048, 1024, 768
HEADS, DIM_HEAD = 16, 64
INNER = HEADS * DIM_HEAD  # 1024
NUM_FRAMES, NUM_PATCHES = 8, 256
LN_EPS = 1e-5
N_CORES = 8
TOK = B * T // N_CORES          # 1024 tokens per core
NB = TOK // NUM_PATCHES         # 4 frame-blocks per core
BLK = NUM_PATCHES               # 256
DT = Q_DIM // 128               # 8 q-dim partition tiles
DKT = KV_DIM // 128             # 6 kv-dim partition tiles
SCALE = DIM_HEAD ** -0.5        # 0.125

_CACHE = {}


def _patch_tile_drain():
    """This walrus build rejects >1 sync-wait on a Drain CTRL instruction.
    Split the Tile end-of-context drain waits across single-wait NOPs."""
    import concourse.tile as tile
    from concourse import mybir
    from concourse.vector_clock import ScopedClock

    if getattr(tile.TileContext, "_drain_patched", False):
        return

    def _drain_and_barrier(self, tick_clock, wait_clock):
        nc = self.nc
        probe = nc.sync.nop(nofuse=True)
        wait_clock.add_sem_waits(
            probe.ins, ScopedClock({None: tick_clock.global_clock})
        )
        si = probe.ins.sync_info
        waits = list(si.on_wait) if si is not None else []
        if waits:
            probe.ins.sync_info = mybir.SyncInfo(on_wait=[waits[0]], on_update=[])
            for w in waits[1:]:
                n = nc.sync.nop(nofuse=True)
                n.ins.sync_info = mybir.SyncInfo(on_wait=[w], on_update=[])
        nc.sync.drain()
        nc.all_engine_barrier()
        assert self.sems is not None
        popped = nc._tile_sem_poison_stack.pop()
        assert popped is self._sem_poison
        nc.clear_and_free_semaphores(list(self.sems.allocated().values()))

    tile.TileContext._drain_and_barrier = _drain_and_barrier
    tile.TileContext._drain_patched = True


def _split_multi_waits(nc, mybir, max_waits=1):
    """This walrus build accepts at most one sync-wait per instruction.
    Move extra waits onto single-wait NOPs inserted just before, on the
    same engine (sound: same-engine program order is preserved)."""
    ctr = [0]
    for fn in nc.m.functions:
        for blk in fn.blocks:
            new = []
            changed = False
            for inst in blk.instructions:
                si = inst.sync_info
                waits = list(si.on_wait) if si is not None else []
                if len(waits) > max_waits:
                    changed = True
                    for w in waits[:-max_waits]:
                        ctr[0] += 1
                        new.append(mybir.InstNoOp(
                            name=f"I-waitsplit-{ctr[0]}",
                            engine=inst.engine,
                            sync_info=mybir.SyncInfo(on_wait=[w], on_update=[]),
                        ))
                    inst.sync_info = mybir.SyncInfo(
                        on_wait=waits[-max_waits:],
                        on_update=list(si.on_update),
                    )
                new.append(inst)
            if changed:
                blk.instructions = new


def _build_nc(has_beta):
    import contextlib

    import concourse.bass as bass
    import concourse.tile as tile
    from concourse import mybir

    _patch_tile_drain()

    f32 = mybir.dt.float32
    f32r = mybir.dt.float32r
    bf16 = mybir.dt.bfloat16

    nc = bass.Bass()

    # All big inputs are host-pre-tiled to [128, ...] so every DMA line is
    # contiguous per partition.
    xT = nc.declare_dram_parameter("xT", [128, DT * TOK], bf16, isOutput=False)
    ctxT = nc.declare_dram_parameter("ctxT", [128, DKT * TOK], bf16, isOutput=False)
    wq = nc.declare_dram_parameter("wq", [128, DT * INNER], bf16, isOutput=False)
    wk = nc.declare_dram_parameter("wk", [128, DKT * INNER], bf16, isOutput=False)
    wv = nc.declare_dram_parameter("wv", [128, DKT * INNER], bf16, isOutput=False)
    wo = nc.declare_dram_parameter("wo", [128, DT * Q_DIM], bf16, isOutput=False)
    wsum_neg = nc.declare_dram_parameter("wsum_neg", [1, INNER], f32r, isOutput=False)
    bias_q = nc.declare_dram_parameter("bias_q", [1, INNER], f32r, isOutput=False)
    bo = nc.declare_dram_parameter("bo", [1, Q_DIM], f32, isOutput=False)
    ones_in = nc.declare_dram_parameter("ones_in", [1, 128], f32r, isOutput=False)
    y = nc.declare_dram_parameter("y", [TOK, Q_DIM], bf16, isOutput=True)

    Hq = TOK // 2   # 512-token half
    Qu = TOK // 4   # 256-token quarter

    with tile.TileContext(nc) as tc:
        with contextlib.ExitStack() as ctx:
            singles = ctx.enter_context(tc.tile_pool(name="singles", bufs=1))
            xsq_pool = ctx.enter_context(tc.tile_pool(name="xsq", bufs=4))
            pt_pool = ctx.enter_context(tc.tile_pool(name="pt", bufs=18))
            osb_pool = ctx.enter_context(tc.tile_pool(name="osb", bufs=4))
            ot_pool = ctx.enter_context(tc.tile_pool(name="ot", bufs=4))
            rc_pool = ctx.enter_context(tc.tile_pool(name="rc", bufs=4))
            tmp_pool = ctx.enter_context(tc.tile_pool(name="tmp", bufs=2))
            y_pool = ctx.enter_context(tc.tile_pool(name="y", bufs=2))
            ps_proj = ctx.enter_context(
                tc.tile_pool(name="ps_proj", bufs=2, space="PSUM")
            )
            ps_st = ctx.enter_context(tc.tile_pool(name="ps_st", bufs=4, space="PSUM"))
            ps_av = ctx.enter_context(tc.tile_pool(name="ps_av", bufs=2, space="PSUM"))

            # ---- resident tiles -------------------------------------------
            xT_sb = singles.tile([128, DT, TOK], bf16)
            ctxT_sb = singles.tile([128, DKT, TOK], bf16)
            wq_sb = singles.tile([128, DT, INNER], bf16)
            wk_sb = singles.tile([128, DKT, INNER], bf16)
            wv_sb = singles.tile([128, DKT, INNER], bf16)
            wo_sb = singles.tile([128, DT, Q_DIM], bf16)
            V_all = singles.tile([128, NB * 2, HEADS * 65], bf16)
            QT_all = singles.tile([128, DT, TOK], bf16)
            KT_all = singles.tile([128, DT, TOK], bf16)
            mu_sb = singles.tile([1, TOK], f32r)
            var_sb = singles.tile([1, TOK], f32r)
            rstd_sb = singles.tile([1, TOK], f32r)
            if has_beta:
                rinv_sb = singles.tile([1, TOK], f32r)
                biasq_sb = singles.tile([1, INNER], f32r)
            mu_bc = singles.tile([128, TOK], f32)
            rbc = singles.tile([128, TOK], f32)
            bo_sb = singles.tile([128, Q_DIM], f32)
            ones_col = singles.tile([1, 128], f32r)
            ones_inv_d = singles.tile([128, 1], bf16)
            eps_sb = singles.tile([1, 1], f32)

            xT_v = xT.rearrange("p (a t) -> p a t", t=TOK)
            ctxT_v = ctxT.rearrange("p (a t) -> p a t", t=TOK)
            wq_v = wq.rearrange("p (a j) -> p a j", j=INNER)
            wk_v = wk.rearrange("p (a j) -> p a j", j=INNER)
            wv_v = wv.rearrange("p (a j) -> p a j", j=INNER)
            wo_v = wo.rearrange("p (a j) -> p a j", j=Q_DIM)

            # ---- DMA issue, first-use order per queue ---------------------
            # sync (hw queue A): xT token-quarters, then wq
            for qq in range(4):
                sl = slice(qq * Qu, (qq + 1) * Qu)
                nc.sync.dma_start(out=xT_sb[:, :, sl], in_=xT_v[:, :, sl])
            nc.sync.dma_start(out=wq_sb, in_=wq_v)
            # scalar (hw queue B): ctx half 0, wv, ctx half 1, small params
            nc.scalar.dma_start(out=ctxT_sb[:, :, 0:Hq], in_=ctxT_v[:, :, 0:Hq])
            nc.scalar.dma_start(out=wv_sb, in_=wv_v)
            nc.scalar.dma_start(out=ctxT_sb[:, :, Hq:], in_=ctxT_v[:, :, Hq:])
            nc.scalar.dma_start(out=bo_sb, in_=bo[:, :].to_broadcast([128, Q_DIM]))
            nc.scalar.dma_start(out=ones_col, in_=ones_in[:, :])
            if has_beta:
                nc.scalar.dma_start(out=biasq_sb, in_=bias_q[:, :])
            # gpsimd (software DGE): wk, wo
            nc.gpsimd.dma_start(out=wk_sb, in_=wk_v)
            nc.gpsimd.dma_start(out=wo_sb, in_=wo_v)

            # ---- constants ------------------------------------------------
            nc.vector.memset(ones_inv_d, 1.0 / Q_DIM)
            nc.vector.memset(eps_sb, LN_EPS)
            nc.gpsimd.memset(
                V_all.rearrange("p t (h c) -> p t h c", c=65)[:, :, :, 64:65], 1.0
            )

            Exp = mybir.ActivationFunctionType.Exp
            Sqrt = mybir.ActivationFunctionType.Sqrt
            Square = mybir.ActivationFunctionType.Square

            # ---- phase emitters -------------------------------------------
            def ln_quarter(qq):
                sl = slice(qq * Qu, (qq + 1) * Qu)
                xsqs = []
                for kt in range(DT):
                    xsq = xsq_pool.tile([128, Qu], bf16, tag="xsq", name="xsq")
                    nc.scalar.activation(xsq, xT_sb[:, kt, sl], Square)
                    xsqs.append(xsq)
                mups = ps_st.tile([1, Qu], f32, tag="stps", name="mups")
                for kt in range(DT):
                    nc.tensor.matmul(
                        mups, ones_inv_d, xT_sb[:, kt, sl],
                        start=(kt == 0), stop=(kt == DT - 1),
                    )
                nc.vector.tensor_copy(mu_sb[:, sl], mups)
                sqps = ps_st.tile([1, Qu], f32, tag="stps", name="sqps")
                for kt in range(DT):
                    nc.tensor.matmul(
                        sqps, ones_inv_d, xsqs[kt],
                        start=(kt == 0), stop=(kt == DT - 1),
                    )
                nc.vector.tensor_copy(var_sb[:, sl], sqps)  # mean(x^2)

            def ln_finalize(half):
                sl = slice(half * Hq, (half + 1) * Hq)
                musq = tmp_pool.tile([1, Hq], f32, tag="musq")
                nc.vector.tensor_mul(musq, mu_sb[:, sl], mu_sb[:, sl])
                nc.vector.tensor_sub(var_sb[:, sl], var_sb[:, sl], musq)
                sqv = tmp_pool.tile([1, Hq], f32, tag="sqv")
                nc.scalar.activation(sqv, var_sb[:, sl], Sqrt, bias=eps_sb)
                if has_beta:
                    nc.vector.tensor_copy(rinv_sb[:, sl], sqv)
                with nc.allow_low_precision(reason="fp32r rounding for PE"):
                    nc.vector.reciprocal(out=rstd_sb[:, sl], in_=sqv)

            def bcasts(half):
                sl = slice(half * Hq, (half + 1) * Hq)
                rbcps = ps_st.tile([128, Hq], f32, tag="stps", name="rbcps")
                nc.tensor.matmul(
                    rbcps, ones_col, rstd_sb[:, sl], start=True, stop=True
                )
                nc.vector.tensor_copy(rbc[:, sl], rbcps)
                mbps = ps_st.tile([128, Hq], f32, tag="stps", name="mbps")
                nc.tensor.matmul(
                    mbps, ones_col, mu_sb[:, sl], start=True, stop=True
                )
                nc.vector.tensor_copy(mu_bc[:, sl], mbps)

            def subs(half):
                sl = slice(half * Hq, (half + 1) * Hq)
                for kt in range(DT):
                    nc.vector.tensor_sub(
                        xT_sb[:, kt, sl], xT_sb[:, kt, sl], mu_bc[:, sl]
                    )

            def k_proj_half(half):
                sl = slice(half * Hq, (half + 1) * Hq)
                for jt in range(DT):
                    js = jt * 128
                    kps = ps_proj.tile([128, Hq], f32, tag="proj", name="kps")
                    for kt in range(DKT):
                        nc.tensor.matmul(
                            kps, wk_sb[:, kt, js:js + 128], ctxT_sb[:, kt, sl],
                            start=(kt == 0), stop=(kt == DKT - 1),
                        )
                    nc.scalar.copy(KT_all[:, jt, sl], kps)

            def v_proj_half(half):
                for tokt in range(half * NB, (half + 1) * NB):
                    cs = tokt * 128
                    for jn in range(2):
                        vps = ps_proj.tile([128, 512], f32, tag="proj", name="vps")
                        for kt in range(DKT):
                            nc.tensor.matmul(
                                vps,
                                ctxT_sb[:, kt, cs:cs + 128],
                                wv_sb[:, kt, jn * 512:(jn + 1) * 512],
                                start=(kt == 0), stop=(kt == DKT - 1),
                            )
                        nc.vector.tensor_copy(
                            V_all.rearrange("p t (h c) -> p t h c", c=65)[
                                :, tokt, jn * 8:(jn + 1) * 8, 0:64
                            ],
                            vps.rearrange("p (h c) -> p h c", c=64),
                        )

            # Q projection for one (jt, half), emitted whole (startup) or as
            # per-matmul closures (pipeline big-stream donors).
            def q_proj_tile(jt, half):
                for mm in q_proj_mms(half, [jt]):
                    mm()

            def q_proj_mms(half, jts):
                sl = slice(half * Hq, (half + 1) * Hq)
                out = []
                for jt in jts:
                    js = jt * 128
                    st = {}
                    for kt in range(DT):
                        def mm(jt=jt, js=js, kt=kt, st=st):
                            if kt == 0:
                                st["ps"] = ps_proj.tile([128, Hq], f32, tag="proj", name="qps")
                            nc.tensor.matmul(
                                st["ps"], wq_sb[:, kt, js:js + 128],
                                xT_sb[:, kt, sl],
                                start=(kt == 0),
                                stop=(kt == DT - 1 and not has_beta),
                            )
                            if kt == DT - 1:
                                if has_beta:
                                    nc.tensor.matmul(
                                        st["ps"], biasq_sb[:, js:js + 128],
                                        rinv_sb[:, sl], start=False, stop=True,
                                    )
                                nc.vector.tensor_mul(
                                    QT_all[:, jt, sl], st["ps"], rbc[:, sl]
                                )
                        out.append(mm)
                return out

            # ---- startup: DMA-chased projections --------------------------
            ln_quarter(0)
            ln_quarter(1)
            ln_finalize(0)
            k_proj_half(0)
            ln_quarter(2)
            bcasts(0)
            subs(0)
            v_proj_half(0)
            ln_quarter(3)
            ln_finalize(1)
            k_proj_half(1)
            bcasts(1)
            subs(1)
            v_proj_half(1)
            for jt in range(DT):
                q_proj_tile(jt, 0)

            # ---- software-pipelined attention -----------------------------
            # slot s: scores(s) | AV(s-1) | bigs: Q-h1 (slots 0-1) or
            # out-proj(s-2); every small AV matmul immediately follows a
            # big-stream matmul so its LDWEIGHTS hides.
            pts = {}      # (b, hg, hh) -> exp(S^T) tile [128, 512]
            osb_t = {}    # (b, t1t) -> O tile [128, INNER]
            ot_t = {}     # (b, t1t) -> O^T tile [128, DT, 128]
            y_t = {}      # mtl parity -> y SBUF tile

            def score_mms(b, hg, sts):
                ts = b * BLK
                out = []
                for t2t in range(2):
                    for hh in range(2):
                        h = hg * 2 + hh
                        jt, po = h // 2, (h % 2) * 64
                        out.append((lambda t2t=t2t, hh=hh, jt=jt, po=po: nc.tensor.matmul(
                            sts[hh][:, t2t * BLK:(t2t + 1) * BLK],
                            KT_all[po:po + 64, jt,
                                   ts + t2t * 128:ts + (t2t + 1) * 128],
                            QT_all[po:po + 64, jt, ts:ts + BLK],
                            start=True, stop=True,
                        )))
                return out

            def av_mms(b, hg, avp):
                t1t, i = hg % 2, hg // 2
                out = []
                for hl in range(4):
                    h = 4 * i + hl
                    hgg, hh = h // 2, h % 2
                    pt = pts[(b, hgg, hh)]
                    for t2t in range(2):
                        out.append((lambda hl=hl, h=h, pt=pt, t2t=t2t: nc.tensor.matmul(
                            avp[:, hl, :],
                            pt[:, t2t * BLK + t1t * 128:t2t * BLK + t1t * 128 + 128],
                            V_all[:, 2 * b + t2t, h * 65:(h + 1) * 65],
                            start=(t2t == 0), stop=(t2t == 1),
                        )))
                return out

            def oproj_mms(b):
                out = []
                for mtl in range(2):
                    for on in range(2):
                        st = {}
                        for kt in range(DT):
                            def mm(mtl=mtl, on=on, kt=kt, st=st):
                                if kt == 0 and on == 0:
                                    y_t[mtl % 2] = y_pool.tile(
                                        [128, Q_DIM], bf16, tag="y", name="y"
                                    )
                                if kt == 0:
                                    st["ps"] = ps_proj.tile(
                                        [128, 512], f32, tag="proj", name="yps"
                                    )
                                nc.tensor.matmul(
                                    st["ps"],
                                    ot_t[(b, mtl)][:, kt, :],
                                    wo_sb[:, kt, on * 512:(on + 1) * 512],
                                    start=(kt == 0), stop=(kt == DT - 1),
                                )
                                if kt == DT - 1:
                                    nc.vector.tensor_add(
                                        y_t[mtl % 2][:, on * 512:(on + 1) * 512],
                                        st["ps"],
                                        bo_sb[:, on * 512:(on + 1) * 512],
                                    )
                                    if on == 1:
                                        ms = (2 * b + mtl) * 128
                                        nc.gpsimd.dma_start(
                                            out=y[ms:ms + 128, :], in_=y_t[mtl % 2]
                                        )
                            out.append(mm)
                return out

            for slot in range(NB + 2):
                sb = slot if slot < NB else None
                ab = slot - 1 if 1 <= slot <= NB else None
                if slot == 0:
                    bigs = q_proj_mms(1, range(0, 4))
                elif slot == 1:
                    bigs = q_proj_mms(1, range(4, 8))
                else:
                    bigs = oproj_mms(slot - 2)

                if sb is None and ab is None:
                    for g in bigs:
                        g()
                    continue

                big_iter = iter(bigs)

                for hg in range(8):
                    s_list = []
                    if sb is not None:
                        sts = [
                            ps_st.tile([128, 512], f32, tag="stps", name=f"st{i}")
                            for i in range(2)
                        ]
                        s_list = score_mms(sb, hg, sts)
                    a_list = []
                    if ab is not None:
                        avp = ps_av.tile([128, 4, 65], f32, tag="avps", name="avp")
                        a_list = av_mms(ab, hg, avp)

                    # 1:1 big:small — every AV matmul follows a stream-rich
                    # matmul so its stationary load is hidden.
                    for u in range(4):
                        if s_list:
                            s_list[u]()
                        if a_list:
                            a_list[2 * u]()
                        g = next(big_iter, None)
                        if g is not None:
                            g()
                        if a_list:
                            a_list[2 * u + 1]()

                    if sb is not None:
                        for hh in range(2):
                            pt = pt_pool.tile([128, 512], bf16, tag="pt", name="pt")
                            nc.scalar.activation(pt, sts[hh], Exp, scale=SCALE)
                            pts[(sb, hg, hh)] = pt

                    if ab is not None:
                        t1t, i = hg % 2, hg // 2
                        if i == 0:
                            osb_t[(ab, t1t)] = osb_pool.tile(
                                [128, INNER], bf16, tag="osb", name=f"osb{t1t}"
                            )
                        rc = rc_pool.tile([128, 4], f32, tag="rc", name="rc")
                        nc.vector.reciprocal(
                            out=rc,
                            in_=avp[:, :, 64:65].rearrange("p h c -> p (h c)"),
                        )
                        nc.vector.tensor_mul(
                            osb_t[(ab, t1t)].rearrange("p (h c) -> p h c", c=64)[
                                :, 4 * i:4 * i + 4, :
                            ],
                            avp[:, :, 0:64],
                            rc.rearrange("p (h o) -> p h o", o=1).to_broadcast(
                                [128, 4, 64]
                            ),
                        )
                        if hg >= 6:
                            ot = ot_pool.tile([128, DT, 128], bf16, tag="ot", name="ot")
                            nc.sync.dma_start_transpose(ot, osb_t[(ab, t1t)])
                            ot_t[(ab, t1t)] = ot

                # leftover bigs of this slot (shouldn't happen, but flush)
                for g in big_iter:
                    g()

    _split_multi_waits(nc, mybir)
    return nc


def _expected_mask():
    fid = np.repeat(np.arange(NUM_FRAMES), NUM_PATCHES)
    return (fid[:, None] == fid[None, :])[None, None]


def _reference_fallback(x, context, ln_gamma, ln_beta, Wq, Wkv, Wo, bo, mask):
    """Pure-numpy fallback for a non-block-diagonal mask (correctness only)."""
    x64 = x.astype(np.float64)
    mu = x64.mean(-1, keepdims=True)
    var = ((x64 - mu) ** 2).mean(-1, keepdims=True)
    xn = (x64 - mu) / np.sqrt(var + LN_EPS) * ln_gamma + ln_beta
    q = xn @ Wq.astype(np.float64)
    kv = context.astype(np.float64) @ Wkv.astype(np.float64)
    k, v = kv[..., :INNER], kv[..., INNER:]
    sh = lambda t: t.reshape(B, T, HEADS, DIM_HEAD).transpose(0, 2, 1, 3)
    q, k, v = sh(q), sh(k), sh(v)
    dots = np.einsum("bhnd,bhmd->bhnm", q, k) * SCALE
    dots = np.where(mask, dots, -np.inf)
    dots -= dots.max(-1, keepdims=True)
    e = np.exp(dots)
    attn = e / e.sum(-1, keepdims=True)
    out = np.einsum("bhnm,bhmd->bhnd", attn, v)
    out = out.transpose(0, 2, 1, 3).reshape(B, T, INNER)
    return (out @ Wo.astype(np.float64) + bo).astype(np.float32)


def _tile128(a):
    """[R, C] -> [128, (R/128)*C] partition-major pre-tiling for one-shot
    contiguous DMA into an SBUF [128, R/128, C] tile."""
    r, c = a.shape
    return np.ascontiguousarray(
        a.reshape(r // 128, 128, c).transpose(1, 0, 2).reshape(128, -1)
    )


def _prep_in_maps(x, context, ln_gamma, ln_beta, Wq, Wkv, Wo, bo):
    import ml_dtypes

    bf = ml_dtypes.bfloat16
    wq_eff = (ln_gamma[:, None] * Wq).astype(np.float32)
    wsum_neg = (-wq_eff.sum(axis=0, dtype=np.float64)).astype(np.float32)[None, :]
    bias_q = (ln_beta @ Wq).astype(np.float32)[None, :]
    wq_t = _tile128(wq_eff.astype(bf))
    wk_t = _tile128(np.ascontiguousarray(Wkv[:, :INNER]).astype(bf))
    wv_t = _tile128(np.ascontiguousarray(Wkv[:, INNER:]).astype(bf))
    wo_t = _tile128(Wo.astype(bf))
    bo2 = bo.astype(np.float32)[None, :]
    ones128 = np.ones((1, 128), np.float32)

    x_flat = x.reshape(B * T, Q_DIM)
    c_flat = context.reshape(B * T, KV_DIM)
    in_maps = []
    for c in range(N_CORES):
        sl = slice(c * TOK, (c + 1) * TOK)
        xT_t = _tile128(np.ascontiguousarray(x_flat[sl].T.astype(bf)))
        ctxT_t = _tile128(np.ascontiguousarray(c_flat[sl].T.astype(bf)))
        in_maps.append({
            "xT": xT_t, "ctxT": ctxT_t,
            "wq": wq_t, "wk": wk_t, "wv": wv_t, "wo": wo_t,
            "wsum_neg": wsum_neg, "bias_q": bias_q, "bo": bo2,
            "ones_in": ones128,
        })
    return in_maps


def _run(inputs, trace=False):
    from concourse.bass_utils import run_bass_kernel_spmd

    has_beta = bool(np.any(np.asarray(inputs["ln_beta"])))
    key = ("nc", has_beta)
    if key not in _CACHE:
        _CACHE[key] = _build_nc(has_beta)
    nc = _CACHE[key]
    in_maps = _prep_in_maps(
        inputs["x"], inputs["context"], inputs["ln_gamma"], inputs["ln_beta"],
        inputs["Wq"], inputs["Wkv"], inputs["Wo"], inputs["bo"],
    )
    res = run_bass_kernel_spmd(nc, in_maps, list(range(N_CORES)), trace=trace)
    ys = [np.asarray(res.results[c]["y"]).astype(np.float32)
          for c in range(N_CORES)]
    out = np.concatenate(ys, axis=0)
    return out.reshape(B, T, Q_DIM), res


def kernel(x, context, ln_gamma, ln_beta, Wq, Wkv, Wo, bo, mask):
    mask = np.asarray(mask)
    if not np.array_equal(mask, _expected_mask()):
        return _reference_fallback(
            np.asarray(x), np.asarray(context), np.asarray(ln_gamma),
            np.asarray(ln_beta), np.asarray(Wq), np.asarray(Wkv),
            np.asarray(Wo), np.asarray(bo), mask,
        )
    inputs = dict(x=np.asarray(x), context=np.asarray(context),
                  ln_gamma=np.asarray(ln_gamma), ln_beta=np.asarray(ln_beta),
                  Wq=np.asarray(Wq), Wkv=np.asarray(Wkv), Wo=np.asarray(Wo),
                  bo=np.asarray(bo))
    out, _ = _run(inputs, trace=False)
    return out


def _install_profiling_shims():
    """Enable the NTFF profile path under axon in this trimmed container:
    provide the antenv.axon_hooks registry and stub the artifact upload."""
    import sys
    import types

    if "antenv.axon_hooks" not in sys.modules:
        import antenv

        mod = types.ModuleType("antenv.axon_hooks")
        mod._hook = None

        def set_axon_ntff_profile_hook(h):
            mod._hook = h

        def get_axon_ntff_profile_hook():
            return mod._hook

        mod.set_axon_ntff_profile_hook = set_axon_ntff_profile_hook
        mod.get_axon_ntff_profile_hook = get_axon_ntff_profile_hook
        sys.modules["antenv.axon_hooks"] = mod
        antenv.axon_hooks = mod

    mod = sys.modules["antenv.axon_hooks"]
    if mod._hook is None:
        from trn_agent_boot.trn_boot import _ntff_profile_via_ctypes

        mod.set_axon_ntff_profile_hook(
            _ntff_profile_via_ctypes("/opt/axon/libaxon_pjrt.so")
        )

    from concourse import bass_utils

    if not getattr(bass_utils, "_upload_stubbed", False):
        bass_utils.upload_artifacts = lambda tmpdir: tmpdir
        bass_utils._upload_stubbed = True


def kernel_traced(**inputs):
    """Like kernel() but returns (out, BassKernelResults) with profiling."""
    _install_profiling_shims()
    out, res = _run(inputs, trace=True)
    return out, res


# revision 11
# speedup vs baseline: 1.0610x; 1.0610x over previous
"""Trainium2 Bass kernel for CrossAttentionInjection (block-diagonal frame attention).

Contract: kernel(**inputs) takes FULL unsharded numpy inputs (as produced by
setup_inputs()) and returns the FULL [B, T, Q_DIM] float32 output.

Sharding: the attention mask is block-diagonal over 8 frames x 256 patches, so
the whole module decomposes into 32 independent (batch, frame) blocks of 256
tokens. Each of the 8 cores processes 4 contiguous blocks (1024 tokens of one
batch) with replicated weights -- no collectives.

Per-core pipeline (bf16 matmuls, fp32 PSUM):
  - DMA-chased startup: xT arrives in token quarters on the sync queue while
    ctx/weights stream on the scalar + gpsimd queues in first-use order; LN
    stats / K-proj / V-proj emissions chase the arrivals so the PE never waits
    on a transfer it does not need yet.
  - LN folded into the Q projection (gamma folded into Wq on the host, mean
    subtracted in-place, 1/std applied in the PSUM->SBUF multiply).
  - Attention is software-pipelined across blocks: scores(b) / AV(b-1) /
    out-proj(b-2) matmuls are interleaved 1:1 big:small so every tiny AV
    matmul's 105ns stationary load hides under the previous matmul's stream.
  - AV PSUM tiles hold 4 heads ([128, 4, 65] with a ones column at 64 for the
    softmax denominator); one reciprocal + one broadcast multiply per tile.
  - O -> O^T via XBAR dma_start_transpose (DMA engine), not PE transposes.
  - Output written bf16 (host upcasts), bias added on gpsimd.
"""

import numpy as np

# ---------------------------------------------------------------------------
# Problem constants (hardcoded; kernel.py must be self-contained)
# ---------------------------------------------------------------------------
B, T, Q_DIM, KV_DIM = 4, 2048, 1024, 768
HEADS, DIM_HEAD = 16, 64
INNER = HEADS * DIM_HEAD  # 1024
NUM_FRAMES, NUM_PATCHES = 8, 256
LN_EPS = 1e-5
N_CORES = 8
TOK = B * T // N_CORES          # 1024 tokens per core
NB = TOK // NUM_PATCHES         # 4 frame-blocks per core
BLK = NUM_PATCHES               # 256
DT = Q_DIM // 128               # 8 q-dim partition tiles
DKT = KV_DIM // 128             # 6 kv-dim partition tiles
SCALE = DIM_HEAD ** -0.5        # 0.125

_CACHE = {}


def _patch_tile_drain():
    """This walrus build rejects >1 sync-wait on a Drain CTRL instruction.
    Split the Tile end-of-context drain waits across single-wait NOPs."""
    import concourse.tile as tile
    from concourse import mybir
    from concourse.vector_clock import ScopedClock

    if getattr(tile.TileContext, "_drain_patched", False):
        return

    def _drain_and_barrier(self, tick_clock, wait_clock):
        nc = self.nc
        probe = nc.sync.nop(nofuse=True)
        wait_clock.add_sem_waits(
            probe.ins, ScopedClock({None: tick_clock.global_clock})
        )
        si = probe.ins.sync_info
        waits = list(si.on_wait) if si is not None else []
        if waits:
            probe.ins.sync_info = mybir.SyncInfo(on_wait=[waits[0]], on_update=[])
            for w in waits[1:]:
                n = nc.sync.nop(nofuse=True)
                n.ins.sync_info = mybir.SyncInfo(on_wait=[w], on_update=[])
        nc.sync.drain()
        nc.all_engine_barrier()
        assert self.sems is not None
        popped = nc._tile_sem_poison_stack.pop()
        assert popped is self._sem_poison
        nc.clear_and_free_semaphores(list(self.sems.allocated().values()))

    tile.TileContext._drain_and_barrier = _drain_and_barrier
    tile.TileContext._drain_patched = True


def _split_multi_waits(nc, mybir, max_waits=1):
    """This walrus build accepts at most one sync-wait per instruction.
    Move extra waits onto single-wait NOPs inserted just before, on the
    same engine (sound: same-engine program order is preserved)."""
    ctr = [0]
    for fn in nc.m.functions:
        for blk in fn.blocks:
            new = []
            changed = False
            for inst in blk.instructions:
                si = inst.sync_info
                waits = list(si.on_wait) if si is not None else []
                if len(waits) > max_waits:
                    changed = True
                    for w in waits[:-max_waits]:
                        ctr[0] += 1
                        new.append(mybir.InstNoOp(
                            name=f"I-waitsplit-{ctr[0]}",
                            engine=inst.engine,
                            sync_info=mybir.SyncInfo(on_wait=[w], on_update=[]),
                        ))
                    inst.sync_info = mybir.SyncInfo(
                        on_wait=waits[-max_waits:],
                        on_update=list(si.on_update),
                    )
                new.append(inst)
            if changed:
                blk.instructions = new


def _build_nc(has_beta):
    import contextlib

    import concourse.bass as bass
    import concourse.tile as tile
    from concourse import mybir

    _patch_tile_drain()

    f32 = mybir.dt.float32
    f32r = mybir.dt.float32r
    bf16 = mybir.dt.bfloat16

    nc = bass.Bass()

    # All big inputs are host-pre-tiled to [128, ...] so every DMA line is
    # contiguous per partition.
    xT = nc.declare_dram_parameter("xT", [128, DT * TOK], bf16, isOutput=False)
    ctxT = nc.declare_dram_parameter("ctxT", [128, DKT * TOK], bf16, isOutput=False)
    wq = nc.declare_dram_parameter("wq", [128, DT * INNER], bf16, isOutput=False)
    wk = nc.declare_dram_parameter("wk", [128, DKT * INNER], bf16, isOutput=False)
    wv = nc.declare_dram_parameter("wv", [128, DKT * INNER], bf16, isOutput=False)
    wo = nc.declare_dram_parameter("wo", [128, DT * Q_DIM], bf16, isOutput=False)
    wsum_neg = nc.declare_dram_parameter("wsum_neg", [1, INNER], f32r, isOutput=False)
    bias_q = nc.declare_dram_parameter("bias_q", [1, INNER], f32r, isOutput=False)
    bo = nc.declare_dram_parameter("bo", [1, Q_DIM], f32, isOutput=False)
    ones_in = nc.declare_dram_parameter("ones_in", [1, 128], f32r, isOutput=False)
    y = nc.declare_dram_parameter("y", [TOK, Q_DIM], bf16, isOutput=True)

    Hq = TOK // 2   # 512-token half
    Qu = TOK // 4   # 256-token quarter

    with tile.TileContext(nc) as tc:
        with contextlib.ExitStack() as ctx:
            singles = ctx.enter_context(tc.tile_pool(name="singles", bufs=1))
            xsq_pool = ctx.enter_context(tc.tile_pool(name="xsq", bufs=4))
            pt_pool = ctx.enter_context(tc.tile_pool(name="pt", bufs=18))
            osb_pool = ctx.enter_context(tc.tile_pool(name="osb", bufs=8))
            ot_pool = ctx.enter_context(tc.tile_pool(name="ot", bufs=4))
            rc_pool = ctx.enter_context(tc.tile_pool(name="rc", bufs=4))
            tmp_pool = ctx.enter_context(tc.tile_pool(name="tmp", bufs=2))
            y_pool = ctx.enter_context(tc.tile_pool(name="y", bufs=2))
            ps_proj = ctx.enter_context(
                tc.tile_pool(name="ps_proj", bufs=2, space="PSUM")
            )
            ps_st = ctx.enter_context(tc.tile_pool(name="ps_st", bufs=4, space="PSUM"))
            ps_av = ctx.enter_context(tc.tile_pool(name="ps_av", bufs=2, space="PSUM"))

            # ---- resident tiles -------------------------------------------
            xT_sb = singles.tile([128, DT, TOK], bf16)
            ctxT_sb = singles.tile([128, DKT, TOK], bf16)
            wq_sb = singles.tile([128, DT, INNER], bf16)
            wk_sb = singles.tile([128, DKT, INNER], bf16)
            wv_sb = singles.tile([128, DKT, INNER], bf16)
            wo_sb = singles.tile([128, DT, Q_DIM], bf16)
            V_all = singles.tile([128, NB * 2, HEADS * 65], bf16)
            QT_all = singles.tile([128, DT, TOK], bf16)
            KT_all = singles.tile([128, DT, TOK], bf16)
            mu_sb = singles.tile([1, TOK], f32r)
            var_sb = singles.tile([1, TOK], f32r)
            rstd_sb = singles.tile([1, TOK], f32r)
            if has_beta:
                rinv_sb = singles.tile([1, TOK], f32r)
                biasq_sb = singles.tile([1, INNER], f32r)
            mu_bc = singles.tile([128, TOK], f32)
            rbc = singles.tile([128, TOK], f32)
            bo_sb = singles.tile([128, Q_DIM], f32)
            ones_col = singles.tile([1, 128], f32r)
            ones_inv_d = singles.tile([128, 1], bf16)
            eps_sb = singles.tile([1, 1], f32)

            xT_v = xT.rearrange("p (a t) -> p a t", t=TOK)
            ctxT_v = ctxT.rearrange("p (a t) -> p a t", t=TOK)
            wq_v = wq.rearrange("p (a j) -> p a j", j=INNER)
            wk_v = wk.rearrange("p (a j) -> p a j", j=INNER)
            wv_v = wv.rearrange("p (a j) -> p a j", j=INNER)
            wo_v = wo.rearrange("p (a j) -> p a j", j=Q_DIM)

            # ---- DMA issue, first-use order per queue ---------------------
            # All transfers are kt-contiguous (4-16KB lines per partition).
            # sync (hw queue A): xT in two kt-chunks, then wq
            nc.sync.dma_start(out=xT_sb[:, 0:DT // 2, :], in_=xT_v[:, 0:DT // 2, :])
            nc.sync.dma_start(out=xT_sb[:, DT // 2:, :], in_=xT_v[:, DT // 2:, :])
            nc.sync.dma_start(out=wq_sb, in_=wq_v)
            # scalar (hw queue B): ctx, wv, small params
            nc.scalar.dma_start(out=ctxT_sb, in_=ctxT_v)
            nc.scalar.dma_start(out=wv_sb, in_=wv_v)
            nc.scalar.dma_start(out=bo_sb, in_=bo[:, :].to_broadcast([128, Q_DIM]))
            nc.scalar.dma_start(out=ones_col, in_=ones_in[:, :])
            if has_beta:
                nc.scalar.dma_start(out=biasq_sb, in_=bias_q[:, :])
            # gpsimd (software DGE): wk, wo
            nc.gpsimd.dma_start(out=wk_sb, in_=wk_v)
            nc.gpsimd.dma_start(out=wo_sb, in_=wo_v)

            # ---- constants ------------------------------------------------
            nc.vector.memset(ones_inv_d, 1.0 / Q_DIM)
            nc.vector.memset(eps_sb, LN_EPS)
            nc.gpsimd.memset(
                V_all.rearrange("p t (h c) -> p t h c", c=65)[:, :, :, 64:65], 1.0
            )

            Exp = mybir.ActivationFunctionType.Exp
            Sqrt = mybir.ActivationFunctionType.Sqrt
            Square = mybir.ActivationFunctionType.Square

            # ---- phase emitters -------------------------------------------
            def ln_stats():
                # mu / mean(x^2) psums accumulate over kt so the matmuls can
                # chase the two xT kt-chunk DMAs.
                st_ps = [ps_st.tile([1, Hq], f32, tag="stps", name=f"lnps{i}")
                         for i in range(4)]  # mu_h0, mu_h1, sq_h0, sq_h1
                for kt in range(DT):
                    xsq = xsq_pool.tile([128, TOK], bf16, tag="xsq", name="xsq")
                    nc.scalar.activation(xsq, xT_sb[:, kt, :], Square)
                    for half in range(2):
                        sl = slice(half * Hq, (half + 1) * Hq)
                        nc.tensor.matmul(
                            st_ps[half], ones_inv_d, xT_sb[:, kt, sl],
                            start=(kt == 0), stop=(kt == DT - 1),
                        )
                        nc.tensor.matmul(
                            st_ps[2 + half], ones_inv_d, xsq[:, sl],
                            start=(kt == 0), stop=(kt == DT - 1),
                        )
                for half in range(2):
                    sl = slice(half * Hq, (half + 1) * Hq)
                    nc.vector.tensor_copy(mu_sb[:, sl], st_ps[half])
                    nc.vector.tensor_copy(var_sb[:, sl], st_ps[2 + half])

            def ln_finalize(half):
                sl = slice(half * Hq, (half + 1) * Hq)
                musq = tmp_pool.tile([1, Hq], f32, tag="musq")
                nc.vector.tensor_mul(musq, mu_sb[:, sl], mu_sb[:, sl])
                nc.vector.tensor_sub(var_sb[:, sl], var_sb[:, sl], musq)
                sqv = tmp_pool.tile([1, Hq], f32, tag="sqv")
                nc.scalar.activation(sqv, var_sb[:, sl], Sqrt, bias=eps_sb)
                if has_beta:
                    nc.vector.tensor_copy(rinv_sb[:, sl], sqv)
                with nc.allow_low_precision(reason="fp32r rounding for PE"):
                    nc.vector.reciprocal(out=rstd_sb[:, sl], in_=sqv)

            def bcasts(half):
                sl = slice(half * Hq, (half + 1) * Hq)
                rbcps = ps_st.tile([128, Hq], f32, tag="stps", name="rbcps")
                nc.tensor.matmul(
                    rbcps, ones_col, rstd_sb[:, sl], start=True, stop=True
                )
                nc.vector.tensor_copy(rbc[:, sl], rbcps)
                mbps = ps_st.tile([128, Hq], f32, tag="stps", name="mbps")
                nc.tensor.matmul(
                    mbps, ones_col, mu_sb[:, sl], start=True, stop=True
                )
                nc.vector.tensor_copy(mu_bc[:, sl], mbps)

            def subs(half):
                # SBUF-only op on the otherwise-idle gpsimd engine
                sl = slice(half * Hq, (half + 1) * Hq)
                for kt in range(DT):
                    nc.gpsimd.tensor_sub(
                        xT_sb[:, kt, sl], xT_sb[:, kt, sl], mu_bc[:, sl]
                    )

            def k_proj_half(half):
                sl = slice(half * Hq, (half + 1) * Hq)
                for jt in range(DT):
                    js = jt * 128
                    kps = ps_proj.tile([128, Hq], f32, tag="proj", name="kps")
                    for kt in range(DKT):
                        nc.tensor.matmul(
                            kps, wk_sb[:, kt, js:js + 128], ctxT_sb[:, kt, sl],
                            start=(kt == 0), stop=(kt == DKT - 1),
                        )
                    nc.vector.tensor_copy(KT_all[:, jt, sl], kps)

            def v_proj_half(half):
                for tokt in range(half * NB, (half + 1) * NB):
                    cs = tokt * 128
                    for jn in range(2):
                        vps = ps_proj.tile([128, 512], f32, tag="proj", name="vps")
                        for kt in range(DKT):
                            nc.tensor.matmul(
                                vps,
                                ctxT_sb[:, kt, cs:cs + 128],
                                wv_sb[:, kt, jn * 512:(jn + 1) * 512],
                                start=(kt == 0), stop=(kt == DKT - 1),
                            )
                        nc.vector.tensor_copy(
                            V_all.rearrange("p t (h c) -> p t h c", c=65)[
                                :, tokt, jn * 8:(jn + 1) * 8, 0:64
                            ],
                            vps.rearrange("p (h c) -> p h c", c=64),
                        )

            # Q projection for one (jt, half), emitted whole (startup) or as
            # per-matmul closures (pipeline big-stream donors).
            def q_proj_tile(jt, half):
                for mm in q_proj_mms(half, [jt]):
                    mm()

            def q_proj_mms(half, jts):
                sl = slice(half * Hq, (half + 1) * Hq)
                out = []
                for jt in jts:
                    js = jt * 128
                    st = {}
                    for kt in range(DT):
                        def mm(jt=jt, js=js, kt=kt, st=st):
                            if kt == 0:
                                st["ps"] = ps_proj.tile([128, Hq], f32, tag="proj", name="qps")
                            nc.tensor.matmul(
                                st["ps"], wq_sb[:, kt, js:js + 128],
                                xT_sb[:, kt, sl],
                                start=(kt == 0),
                                stop=(kt == DT - 1 and not has_beta),
                            )
                            if kt == DT - 1:
                                if has_beta:
                                    nc.tensor.matmul(
                                        st["ps"], biasq_sb[:, js:js + 128],
                                        rinv_sb[:, sl], start=False, stop=True,
                                    )
                                nc.vector.tensor_mul(
                                    QT_all[:, jt, sl], st["ps"], rbc[:, sl]
                                )
                        out.append(mm)
                return out

            # ---- startup: DMA-chased projections --------------------------
            ln_stats()
            ln_finalize(0)
            ln_finalize(1)
            k_proj_half(0)
            bcasts(0)
            bcasts(1)
            subs(0)
            subs(1)
            k_proj_half(1)
            v_proj_half(0)
            v_proj_half(1)
            for jt in range(DT):
                q_proj_tile(jt, 0)

            # ---- software-pipelined attention -----------------------------
            # slot s: scores(s) | AV(s-1) | bigs: Q-h1 (slots 0-1) or
            # out-proj(s-2); every small AV matmul immediately follows a
            # big-stream matmul so its LDWEIGHTS hides.
            pts = {}      # (b, hg, hh) -> exp(S^T) tile [128, 512]
            osb_t = {}    # (b, t1t) -> O tile [128, INNER]
            ot_t = {}     # (b, t1t) -> O^T tile [128, DT, 128]
            y_t = {}      # mtl parity -> y SBUF tile

            def score_mms(b, hg, sts):
                ts = b * BLK
                out = []
                for t2t in range(2):
                    for hh in range(2):
                        h = hg * 2 + hh
                        jt, po = h // 2, (h % 2) * 64
                        out.append((lambda t2t=t2t, hh=hh, jt=jt, po=po: nc.tensor.matmul(
                            sts[hh][:, t2t * BLK:(t2t + 1) * BLK],
                            KT_all[po:po + 64, jt,
                                   ts + t2t * 128:ts + (t2t + 1) * 128],
                            QT_all[po:po + 64, jt, ts:ts + BLK],
                            start=True, stop=True,
                        )))
                return out

            def av_mms(b, hg, avp):
                t1t, i = hg % 2, hg // 2
                out = []
                for hl in range(4):
                    h = 4 * i + hl
                    hgg, hh = h // 2, h % 2
                    pt = pts[(b, hgg, hh)]
                    for t2t in range(2):
                        out.append((lambda hl=hl, h=h, pt=pt, t2t=t2t: nc.tensor.matmul(
                            avp[:, hl, :],
                            pt[:, t2t * BLK + t1t * 128:t2t * BLK + t1t * 128 + 128],
                            V_all[:, 2 * b + t2t, h * 65:(h + 1) * 65],
                            start=(t2t == 0), stop=(t2t == 1),
                        )))
                return out

            def oproj_mms(b):
                out = []
                for mtl in range(2):
                    for on in range(2):
                        st = {}
                        for kt in range(DT):
                            def mm(mtl=mtl, on=on, kt=kt, st=st):
                                if kt == 0 and on == 0:
                                    y_t[mtl % 2] = y_pool.tile(
                                        [128, Q_DIM], bf16, tag="y", name="y"
                                    )
                                if kt == 0:
                                    st["ps"] = ps_proj.tile(
                                        [128, 512], f32, tag="proj", name="yps"
                                    )
                                nc.tensor.matmul(
                                    st["ps"],
                                    ot_t[(b, mtl, kt // 4)][:, kt % 4, :],
                                    wo_sb[:, kt, on * 512:(on + 1) * 512],
                                    start=(kt == 0), stop=(kt == DT - 1),
                                )
                                if kt == DT - 1:
                                    nc.vector.tensor_add(
                                        y_t[mtl % 2][:, on * 512:(on + 1) * 512],
                                        st["ps"],
                                        bo_sb[:, on * 512:(on + 1) * 512],
                                    )
                                    if on == 1:
                                        ms = (2 * b + mtl) * 128
                                        nc.gpsimd.dma_start(
                                            out=y[ms:ms + 128, :], in_=y_t[mtl % 2]
                                        )
                            out.append(mm)
                return out

            for slot in range(NB + 2):
                sb = slot if slot < NB else None
                ab = slot - 1 if 1 <= slot <= NB else None
                if slot == 0:
                    bigs = q_proj_mms(1, range(0, 4))
                elif slot == 1:
                    bigs = q_proj_mms(1, range(4, 8))
                else:
                    bigs = oproj_mms(slot - 2)

                if sb is None and ab is None:
                    for g in bigs:
                        g()
                    continue

                big_iter = iter(bigs)

                for hg in range(8):
                    s_list = []
                    if sb is not None:
                        sts = [
                            ps_st.tile([128, 512], f32, tag="stps", name=f"st{i}")
                            for i in range(2)
                        ]
                        s_list = score_mms(sb, hg, sts)
                    a_list = []
                    if ab is not None:
                        avp = ps_av.tile([128, 4, 65], f32, tag="avps", name="avp")
                        a_list = av_mms(ab, hg, avp)

                    # 1:1 big:small — every AV matmul follows a stream-rich
                    # matmul so its stationary load is hidden.
                    for u in range(4):
                        if s_list:
                            s_list[u]()
                        if a_list:
                            a_list[2 * u]()
                        g = next(big_iter, None)
                        if g is not None:
                            g()
                        if a_list:
                            a_list[2 * u + 1]()

                    if sb is not None:
                        for hh in range(2):
                            pt = pt_pool.tile([128, 512], bf16, tag="pt", name="pt")
                            nc.scalar.activation(pt, sts[hh], Exp, scale=SCALE)
                            pts[(sb, hg, hh)] = pt

                    if ab is not None:
                        t1t, i = hg % 2, hg // 2
                        if i % 2 == 0:
                            osb_t[(ab, t1t, i // 2)] = osb_pool.tile(
                                [128, 512], bf16, tag="osb", name=f"osb{t1t}"
                            )
                        rc = rc_pool.tile([128, 4], f32, tag="rc", name="rc")
                        nc.vector.reciprocal(
                            out=rc,
                            in_=avp[:, :, 64:65].rearrange("p h c -> p (h c)"),
                        )
                        nc.vector.tensor_mul(
                            osb_t[(ab, t1t, i // 2)].rearrange(
                                "p (h c) -> p h c", c=64
                            )[:, 4 * (i % 2):4 * (i % 2) + 4, :],
                            avp[:, :, 0:64],
                            rc.rearrange("p (h o) -> p h o", o=1).to_broadcast(
                                [128, 4, 64]
                            ),
                        )
                        if hg in (2, 3, 6, 7):
                            ph = 0 if hg <= 3 else 1
                            ot = ot_pool.tile(
                                [128, 4, 128], bf16, tag="ot", name="ot"
                            )
                            nc.sync.dma_start_transpose(
                                ot, osb_t[(ab, t1t, ph)]
                            )
                            ot_t[(ab, t1t, ph)] = ot

                # leftover bigs of this slot (shouldn't happen, but flush)
                for g in big_iter:
                    g()

    _split_multi_waits(nc, mybir)
    return nc


def _expected_mask():
    fid = np.repeat(np.arange(NUM_FRAMES), NUM_PATCHES)
    return (fid[:, None] == fid[None, :])[None, None]


def _reference_fallback(x, context, ln_gamma, ln_beta, Wq, Wkv, Wo, bo, mask):
    """Pure-numpy fallback for a non-block-diagonal mask (correctness only)."""
    x64 = x.astype(np.float64)
    mu = x64.mean(-1, keepdims=True)
    var = ((x64 - mu) ** 2).mean(-1, keepdims=True)
    xn = (x64 - mu) / np.sqrt(var + LN_EPS) * ln_gamma + ln_beta
    q = xn @ Wq.astype(np.float64)
    kv = context.astype(np.float64) @ Wkv.astype(np.float64)
    k, v = kv[..., :INNER], kv[..., INNER:]
    sh = lambda t: t.reshape(B, T, HEADS, DIM_HEAD).transpose(0, 2, 1, 3)
    q, k, v = sh(q), sh(k), sh(v)
    dots = np.einsum("bhnd,bhmd->bhnm", q, k) * SCALE
    dots = np.where(mask, dots, -np.inf)
    dots -= dots.max(-1, keepdims=True)
    e = np.exp(dots)
    attn = e / e.sum(-1, keepdims=True)
    out = np.einsum("bhnm,bhmd->bhnd", attn, v)
    out = out.transpose(0, 2, 1, 3).reshape(B, T, INNER)
    return (out @ Wo.astype(np.float64) + bo).astype(np.float32)


def _tile128(a):
    """[R, C] -> [128, (R/128)*C] partition-major pre-tiling for one-shot
    contiguous DMA into an SBUF [128, R/128, C] tile."""
    r, c = a.shape
    return np.ascontiguousarray(
        a.reshape(r // 128, 128, c).transpose(1, 0, 2).reshape(128, -1)
    )


def _prep_in_maps(x, context, ln_gamma, ln_beta, Wq, Wkv, Wo, bo):
    import ml_dtypes

    bf = ml_dtypes.bfloat16
    wq_eff = (ln_gamma[:, None] * Wq).astype(np.float32)
    wsum_neg = (-wq_eff.sum(axis=0, dtype=np.float64)).astype(np.float32)[None, :]
    bias_q = (ln_beta @ Wq).astype(np.float32)[None, :]
    wq_t = _tile128(wq_eff.astype(bf))
    wk_t = _tile128(np.ascontiguousarray(Wkv[:, :INNER]).astype(bf))
    wv_t = _tile128(np.ascontiguousarray(Wkv[:, INNER:]).astype(bf))
    wo_t = _tile128(Wo.astype(bf))
    bo2 = bo.astype(np.float32)[None, :]
    ones128 = np.ones((1, 128), np.float32)

    x_flat = x.reshape(B * T, Q_DIM)
    c_flat = context.reshape(B * T, KV_DIM)
    in_maps = []
    for c in range(N_CORES):
        sl = slice(c * TOK, (c + 1) * TOK)
        xT_t = _tile128(np.ascontiguousarray(x_flat[sl].T.astype(bf)))
        ctxT_t = _tile128(np.ascontiguousarray(c_flat[sl].T.astype(bf)))
        in_maps.append({
            "xT": xT_t, "ctxT": ctxT_t,
            "wq": wq_t, "wk": wk_t, "wv": wv_t, "wo": wo_t,
            "wsum_neg": wsum_neg, "bias_q": bias_q, "bo": bo2,
            "ones_in": ones128,
        })
    return in_maps


def _run(inputs, trace=False):
    from concourse.bass_utils import run_bass_kernel_spmd

    has_beta = bool(np.any(np.asarray(inputs["ln_beta"])))
    key = ("nc", has_beta)
    if key not in _CACHE:
        _CACHE[key] = _build_nc(has_beta)
    nc = _CACHE[key]
    in_maps = _prep_in_maps(
        inputs["x"], inputs["context"], inputs["ln_gamma"], inputs["ln_beta"],
        inputs["Wq"], inputs["Wkv"], inputs["Wo"], inputs["bo"],
    )
    res = run_bass_kernel_spmd(nc, in_maps, list(range(N_CORES)), trace=trace)
    ys = [np.asarray(res.results[c]["y"]).astype(np.float32)
          for c in range(N_CORES)]
    out = np.concatenate(ys, axis=0)
    return out.reshape(B, T, Q_DIM), res


def kernel(x, context, ln_gamma, ln_beta, Wq, Wkv, Wo, bo, mask):
    mask = np.asarray(mask)
    if not np.array_equal(mask, _expected_mask()):
        return _reference_fallback(
            np.asarray(x), np.asarray(context), np.asarray(ln_gamma),
            np.asarray(ln_beta), np.asarray(Wq), np.asarray(Wkv),
            np.asarray(Wo), np.asarray(bo), mask,
        )
    inputs = dict(x=np.asarray(x), context=np.asarray(context),
                  ln_gamma=np.asarray(ln_gamma), ln_beta=np.asarray(ln_beta),
                  Wq=np.asarray(Wq), Wkv=np.asarray(Wkv), Wo=np.asarray(Wo),
                  bo=np.asarray(bo))
    out, _ = _run(inputs, trace=False)
    return out


def _install_profiling_shims():
    """Enable the NTFF profile path under axon in this trimmed container:
    provide the antenv.axon_hooks registry and stub the artifact upload."""
    import sys
    import types

    if "antenv.axon_hooks" not in sys.modules:
        import antenv

        mod = types.ModuleType("antenv.axon_hooks")
        mod._hook = None

        def set_axon_ntff_profile_hook(h):
            mod._hook = h

        def get_axon_ntff_profile_hook():
            return mod._hook

        mod.set_axon_ntff_profile_hook = set_axon_ntff_profile_hook
        mod.get_axon_ntff_profile_hook = get_axon_ntff_profile_hook
        sys.modules["antenv.axon_hooks"] = mod
        antenv.axon_hooks = mod

    mod = sys.modules["antenv.axon_hooks"]
    if mod._hook is None:
        from trn_agent_boot.trn_boot import _ntff_profile_via_ctypes

        mod.set_axon_ntff_profile_hook(
            _ntff_profile_via_ctypes("/opt/axon/libaxon_pjrt.so")
        )

    from concourse import bass_utils

    if not getattr(bass_utils, "_upload_stubbed", False):
        bass_utils.upload_artifacts = lambda tmpdir: tmpdir
        bass_utils._upload_stubbed = True


def kernel_traced(**inputs):
    """Like kernel() but returns (out, BassKernelResults) with profiling."""
    _install_profiling_shims()
    out, res = _run(inputs, trace=True)
    return out, res


# revision 16
# speedup vs baseline: 1.0627x; 1.0016x over previous
"""Trainium2 Bass kernel for CrossAttentionInjection (block-diagonal frame attention).

Contract: kernel(**inputs) takes FULL unsharded numpy inputs (as produced by
setup_inputs()) and returns the FULL [B, T, Q_DIM] float32 output.

Sharding: the attention mask is block-diagonal over 8 frames x 256 patches, so
the whole module decomposes into 32 independent (batch, frame) blocks of 256
tokens. Each of the 8 cores processes 4 contiguous blocks (1024 tokens of one
batch) with replicated weights -- no collectives.

Per-core pipeline (bf16 matmuls, fp32 PSUM):
  - DMA-chased startup: xT arrives in token quarters on the sync queue while
    ctx/weights stream on the scalar + gpsimd queues in first-use order; LN
    stats / K-proj / V-proj emissions chase the arrivals so the PE never waits
    on a transfer it does not need yet.
  - LN folded into the Q projection (gamma folded into Wq on the host, mean
    subtracted in-place, 1/std applied in the PSUM->SBUF multiply).
  - Attention is software-pipelined across blocks: scores(b) / AV(b-1) /
    out-proj(b-2) matmuls are interleaved 1:1 big:small so every tiny AV
    matmul's 105ns stationary load hides under the previous matmul's stream.
  - AV PSUM tiles hold 4 heads ([128, 4, 65] with a ones column at 64 for the
    softmax denominator); one reciprocal + one broadcast multiply per tile.
  - O -> O^T via PE transposes, emitted one head-group late so the PE never
    waits on the normalizing vector multiply.
  - Output written bf16 (host upcasts), bias added on gpsimd.
"""

import numpy as np

# ---------------------------------------------------------------------------
# Problem constants (hardcoded; kernel.py must be self-contained)
# ---------------------------------------------------------------------------
B, T, Q_DIM, KV_DIM = 4, 2048, 1024, 768
HEADS, DIM_HEAD = 16, 64
INNER = HEADS * DIM_HEAD  # 1024
NUM_FRAMES, NUM_PATCHES = 8, 256
LN_EPS = 1e-5
N_CORES = 8
TOK = B * T // N_CORES          # 1024 tokens per core
NB = TOK // NUM_PATCHES         # 4 frame-blocks per core
BLK = NUM_PATCHES               # 256
DT = Q_DIM // 128               # 8 q-dim partition tiles
DKT = KV_DIM // 128             # 6 kv-dim partition tiles
SCALE = DIM_HEAD ** -0.5        # 0.125

_CACHE = {}


def _patch_tile_drain():
    """This walrus build rejects >1 sync-wait on a Drain CTRL instruction.
    Split the Tile end-of-context drain waits across single-wait NOPs."""
    import concourse.tile as tile
    from concourse import mybir
    from concourse.vector_clock import ScopedClock

    if getattr(tile.TileContext, "_drain_patched", False):
        return

    def _drain_and_barrier(self, tick_clock, wait_clock):
        nc = self.nc
        probe = nc.sync.nop(nofuse=True)
        wait_clock.add_sem_waits(
            probe.ins, ScopedClock({None: tick_clock.global_clock})
        )
        si = probe.ins.sync_info
        waits = list(si.on_wait) if si is not None else []
        if waits:
            probe.ins.sync_info = mybir.SyncInfo(on_wait=[waits[0]], on_update=[])
            for w in waits[1:]:
                n = nc.sync.nop(nofuse=True)
                n.ins.sync_info = mybir.SyncInfo(on_wait=[w], on_update=[])
        nc.sync.drain()
        nc.all_engine_barrier()
        assert self.sems is not None
        popped = nc._tile_sem_poison_stack.pop()
        assert popped is self._sem_poison
        nc.clear_and_free_semaphores(list(self.sems.allocated().values()))

    tile.TileContext._drain_and_barrier = _drain_and_barrier
    tile.TileContext._drain_patched = True


def _split_multi_waits(nc, mybir, max_waits=1):
    """This walrus build accepts at most one sync-wait per instruction.
    Move extra waits onto single-wait NOPs inserted just before, on the
    same engine (sound: same-engine program order is preserved)."""
    ctr = [0]
    for fn in nc.m.functions:
        for blk in fn.blocks:
            new = []
            changed = False
            for inst in blk.instructions:
                si = inst.sync_info
                waits = list(si.on_wait) if si is not None else []
                if len(waits) > max_waits:
                    changed = True
                    for w in waits[:-max_waits]:
                        ctr[0] += 1
                        new.append(mybir.InstNoOp(
                            name=f"I-waitsplit-{ctr[0]}",
                            engine=inst.engine,
                            sync_info=mybir.SyncInfo(on_wait=[w], on_update=[]),
                        ))
                    inst.sync_info = mybir.SyncInfo(
                        on_wait=waits[-max_waits:],
                        on_update=list(si.on_update),
                    )
                new.append(inst)
            if changed:
                blk.instructions = new


def _build_nc(has_beta):
    import contextlib

    import concourse.bass as bass
    import concourse.tile as tile
    from concourse import mybir
    from concourse.masks import make_identity

    _patch_tile_drain()

    f32 = mybir.dt.float32
    f32r = mybir.dt.float32r
    bf16 = mybir.dt.bfloat16

    nc = bass.Bass()

    # All big inputs are host-pre-tiled to [128, ...] so every DMA line is
    # contiguous per partition.
    xT = nc.declare_dram_parameter("xT", [128, DT * TOK], bf16, isOutput=False)
    ctxT = nc.declare_dram_parameter("ctxT", [128, DKT * TOK], bf16, isOutput=False)
    wq = nc.declare_dram_parameter("wq", [128, DT * INNER], bf16, isOutput=False)
    wk = nc.declare_dram_parameter("wk", [128, DKT * INNER], bf16, isOutput=False)
    wv = nc.declare_dram_parameter("wv", [128, DKT * INNER], bf16, isOutput=False)
    wo = nc.declare_dram_parameter("wo", [128, DT * Q_DIM], bf16, isOutput=False)
    wsum_neg = nc.declare_dram_parameter("wsum_neg", [1, INNER], f32r, isOutput=False)
    bias_q = nc.declare_dram_parameter("bias_q", [1, INNER], f32r, isOutput=False)
    bo = nc.declare_dram_parameter("bo", [1, Q_DIM], f32, isOutput=False)
    ones_in = nc.declare_dram_parameter("ones_in", [1, 128], f32r, isOutput=False)
    y = nc.declare_dram_parameter("y", [TOK, Q_DIM], bf16, isOutput=True)

    Hq = TOK // 2   # 512-token half
    Qu = TOK // 4   # 256-token quarter

    with tile.TileContext(nc) as tc:
        with contextlib.ExitStack() as ctx:
            singles = ctx.enter_context(tc.tile_pool(name="singles", bufs=1))
            xsq_pool = ctx.enter_context(tc.tile_pool(name="xsq", bufs=4))
            pt_pool = ctx.enter_context(tc.tile_pool(name="pt", bufs=18))
            osb_pool = ctx.enter_context(tc.tile_pool(name="osb", bufs=6))
            rc_pool = ctx.enter_context(tc.tile_pool(name="rc", bufs=4))
            tmp_pool = ctx.enter_context(tc.tile_pool(name="tmp", bufs=1))
            y_pool = ctx.enter_context(tc.tile_pool(name="y", bufs=2))
            ps_proj = ctx.enter_context(
                tc.tile_pool(name="ps_proj", bufs=2, space="PSUM")
            )
            ps_st = ctx.enter_context(tc.tile_pool(name="ps_st", bufs=3, space="PSUM"))
            ps_av = ctx.enter_context(tc.tile_pool(name="ps_av", bufs=2, space="PSUM"))
            ps_tr = ctx.enter_context(tc.tile_pool(name="ps_tr", bufs=1, space="PSUM"))

            # ---- resident tiles -------------------------------------------
            xT_sb = singles.tile([128, DT, TOK], bf16)
            ctxT_sb = singles.tile([128, DKT, TOK], bf16)
            wq_sb = singles.tile([128, DT, INNER], bf16)
            wk_sb = singles.tile([128, DKT, INNER], bf16)
            wv_sb = singles.tile([128, DKT, INNER], bf16)
            wo_sb = singles.tile([128, DT, Q_DIM], bf16)
            V_all = singles.tile([128, NB * 2, HEADS * 65], bf16)
            QT_all = singles.tile([128, DT, TOK], bf16)
            KT_all = singles.tile([128, DT, TOK], bf16)
            mu_sb = singles.tile([1, TOK], f32r)
            var_sb = singles.tile([1, TOK], f32r)
            rstd_sb = singles.tile([1, TOK], f32r)
            if has_beta:
                rinv_sb = singles.tile([1, TOK], f32r)
                biasq_sb = singles.tile([1, INNER], f32r)
            mu_bc = singles.tile([128, TOK], bf16)
            rbc = singles.tile([128, TOK], f32)
            bo_sb = singles.tile([128, Q_DIM], f32)
            ones_col = singles.tile([1, 128], f32r)
            ones_inv_d = singles.tile([128, 1], bf16)
            eps_sb = singles.tile([1, 1], f32)
            ident = singles.tile([128, 128], bf16)
            OT_sb = singles.tile([128, DT, TOK], bf16)

            xT_v = xT.rearrange("p (a t) -> p a t", t=TOK)
            ctxT_v = ctxT.rearrange("p (a t) -> p a t", t=TOK)
            wq_v = wq.rearrange("p (a j) -> p a j", j=INNER)
            wk_v = wk.rearrange("p (a j) -> p a j", j=INNER)
            wv_v = wv.rearrange("p (a j) -> p a j", j=INNER)
            wo_v = wo.rearrange("p (a j) -> p a j", j=Q_DIM)

            # ---- DMA issue, first-use order per queue ---------------------
            # All transfers are kt-contiguous (4-16KB lines per partition).
            # K-proj gates the whole schedule, so ctx and wk lead their
            # queues; x (needed by LN, emitted after K) and wq/wo follow.
            nc.sync.dma_start(out=ctxT_sb, in_=ctxT_v)
            nc.sync.dma_start(out=xT_sb[:, 0:DT // 2, :], in_=xT_v[:, 0:DT // 2, :])
            nc.sync.dma_start(out=xT_sb[:, DT // 2:, :], in_=xT_v[:, DT // 2:, :])
            nc.sync.dma_start(out=wq_sb, in_=wq_v)
            nc.scalar.dma_start(out=wk_sb, in_=wk_v)
            nc.scalar.dma_start(out=wv_sb, in_=wv_v)
            nc.scalar.dma_start(out=bo_sb, in_=bo[:, :].to_broadcast([128, Q_DIM]))
            nc.scalar.dma_start(out=ones_col, in_=ones_in[:, :])
            if has_beta:
                nc.scalar.dma_start(out=biasq_sb, in_=bias_q[:, :])
            nc.gpsimd.dma_start(out=wo_sb, in_=wo_v)

            # ---- constants ------------------------------------------------
            nc.vector.memset(ones_inv_d, 1.0 / Q_DIM)
            nc.vector.memset(eps_sb, LN_EPS)
            nc.gpsimd.memset(
                V_all.rearrange("p t (h c) -> p t h c", c=65)[:, :, :, 64:65], 1.0
            )
            make_identity(nc, ident)

            Exp = mybir.ActivationFunctionType.Exp
            Sqrt = mybir.ActivationFunctionType.Sqrt
            Square = mybir.ActivationFunctionType.Square

            # ---- phase emitters -------------------------------------------
            def ln_stats():
                # mu / mean(x^2) psums accumulate over kt so the matmuls can
                # chase the two xT kt-chunk DMAs.
                st_ps = [ps_st.tile([1, Hq], f32, tag="stps", name=f"lnps{i}")
                         for i in range(3)]  # mu_h0, mu_h1, sq_h0
                st_ps.append(ps_tr.tile([1, Hq], f32, tag="trps", name="lnps3"))
                for kt in range(DT):
                    for half in range(2):
                        sl = slice(half * Hq, (half + 1) * Hq)
                        xsq = xsq_pool.tile([128, Hq], bf16, tag="xsq", name="xsq")
                        nc.scalar.activation(xsq, xT_sb[:, kt, sl], Square)
                        nc.tensor.matmul(
                            st_ps[half], ones_inv_d, xT_sb[:, kt, sl],
                            start=(kt == 0), stop=(kt == DT - 1),
                        )
                        nc.tensor.matmul(
                            st_ps[2 + half], ones_inv_d, xsq,
                            start=(kt == 0), stop=(kt == DT - 1),
                        )
                for half in range(2):
                    sl = slice(half * Hq, (half + 1) * Hq)
                    nc.vector.tensor_copy(mu_sb[:, sl], st_ps[half])
                    nc.vector.tensor_copy(var_sb[:, sl], st_ps[2 + half])

            def ln_finalize(half):
                sl = slice(half * Hq, (half + 1) * Hq)
                musq = tmp_pool.tile([1, Hq], f32, tag="musq")
                nc.vector.tensor_mul(musq, mu_sb[:, sl], mu_sb[:, sl])
                nc.vector.tensor_sub(var_sb[:, sl], var_sb[:, sl], musq)
                sqv = tmp_pool.tile([1, Hq], f32, tag="sqv")
                nc.scalar.activation(sqv, var_sb[:, sl], Sqrt, bias=eps_sb)
                if has_beta:
                    nc.vector.tensor_copy(rinv_sb[:, sl], sqv)
                with nc.allow_low_precision(reason="fp32r rounding for PE"):
                    nc.vector.reciprocal(out=rstd_sb[:, sl], in_=sqv)

            def bcasts(half):
                sl = slice(half * Hq, (half + 1) * Hq)
                rbcps = ps_st.tile([128, Hq], f32, tag="stps", name="rbcps")
                nc.tensor.matmul(
                    rbcps, ones_col, rstd_sb[:, sl], start=True, stop=True
                )
                nc.vector.tensor_copy(rbc[:, sl], rbcps)
                mbps = ps_st.tile([128, Hq], f32, tag="stps", name="mbps")
                nc.tensor.matmul(
                    mbps, ones_col, mu_sb[:, sl], start=True, stop=True
                )
                nc.vector.tensor_copy(mu_bc[:, sl], mbps)

            def subs(half):
                # SBUF-only op on the otherwise-idle gpsimd engine
                sl = slice(half * Hq, (half + 1) * Hq)
                for kt in range(DT):
                    nc.gpsimd.tensor_sub(
                        xT_sb[:, kt, sl], xT_sb[:, kt, sl], mu_bc[:, sl]
                    )

            def k_proj_half(half):
                sl = slice(half * Hq, (half + 1) * Hq)
                for jt in range(DT):
                    js = jt * 128
                    kps = ps_proj.tile([128, Hq], f32, tag="proj", name="kps")
                    for kt in range(DKT):
                        nc.tensor.matmul(
                            kps, wk_sb[:, kt, js:js + 128], ctxT_sb[:, kt, sl],
                            start=(kt == 0), stop=(kt == DKT - 1),
                        )
                    nc.vector.tensor_copy(KT_all[:, jt, sl], kps)

            def v_proj_half(half):
                for tokt in range(half * NB, (half + 1) * NB):
                    cs = tokt * 128
                    for jn in range(2):
                        vps = ps_proj.tile([128, 512], f32, tag="proj", name="vps")
                        for kt in range(DKT):
                            nc.tensor.matmul(
                                vps,
                                ctxT_sb[:, kt, cs:cs + 128],
                                wv_sb[:, kt, jn * 512:(jn + 1) * 512],
                                start=(kt == 0), stop=(kt == DKT - 1),
                            )
                        nc.vector.tensor_copy(
                            V_all.rearrange("p t (h c) -> p t h c", c=65)[
                                :, tokt, jn * 8:(jn + 1) * 8, 0:64
                            ],
                            vps.rearrange("p (h c) -> p h c", c=64),
                        )

            # Q projection for one (jt, half), emitted whole (startup) or as
            # per-matmul closures (pipeline big-stream donors).
            def q_proj_tile(jt, half):
                for mm in q_proj_mms(half, [jt]):
                    mm()

            def q_proj_mms(half, jts):
                sl = slice(half * Hq, (half + 1) * Hq)
                out = []
                for jt in jts:
                    js = jt * 128
                    st = {}
                    for kt in range(DT):
                        def mm(jt=jt, js=js, kt=kt, st=st):
                            if kt == 0:
                                st["ps"] = ps_proj.tile([128, Hq], f32, tag="proj", name="qps")
                            nc.tensor.matmul(
                                st["ps"], wq_sb[:, kt, js:js + 128],
                                xT_sb[:, kt, sl],
                                start=(kt == 0),
                                stop=(kt == DT - 1 and not has_beta),
                            )
                            if kt == DT - 1:
                                if has_beta:
                                    nc.tensor.matmul(
                                        st["ps"], biasq_sb[:, js:js + 128],
                                        rinv_sb[:, sl], start=False, stop=True,
                                    )
                                nc.vector.tensor_mul(
                                    QT_all[:, jt, sl], st["ps"], rbc[:, sl]
                                )
                        out.append(mm)
                return out

            # ---- startup: DMA-chased projections --------------------------
            k_proj_half(0)
            k_proj_half(1)
            ln_stats()
            ln_finalize(0)
            ln_finalize(1)
            bcasts(0)
            bcasts(1)
            subs(0)
            subs(1)
            v_proj_half(0)
            v_proj_half(1)
            for jt in range(DT):
                q_proj_tile(jt, 0)

            # ---- software-pipelined attention -----------------------------
            # slot s: scores(s) | AV(s-1) | bigs: Q-h1 (slots 0-1) or
            # out-proj(s-2); every small AV matmul immediately follows a
            # big-stream matmul so its LDWEIGHTS hides.
            pts = {}      # (b, hg, hh) -> exp(S^T) tile [128, 512]
            osb_t = {}    # (b, t1t, ph) -> O piece [128, 512]
            y_t = {}      # mtl parity -> y SBUF tile
            tr_pend = []  # delayed PE-transpose closures (one hg late)

            def score_mms(b, hg, sts):
                ts = b * BLK
                out = []
                for t2t in range(2):
                    for hh in range(2):
                        h = hg * 2 + hh
                        jt, po = h // 2, (h % 2) * 64
                        out.append((lambda t2t=t2t, hh=hh, jt=jt, po=po: nc.tensor.matmul(
                            sts[hh][:, t2t * BLK:(t2t + 1) * BLK],
                            KT_all[po:po + 64, jt,
                                   ts + t2t * 128:ts + (t2t + 1) * 128],
                            QT_all[po:po + 64, jt, ts:ts + BLK],
                            start=True, stop=True,
                        )))
                return out

            def av_mms(b, hg, avp):
                t1t, i = hg % 2, hg // 2
                out = []
                for hl in range(4):
                    h = 4 * i + hl
                    hgg, hh = h // 2, h % 2
                    pt = pts[(b, hgg, hh)]
                    for t2t in range(2):
                        out.append((lambda hl=hl, h=h, pt=pt, t2t=t2t: nc.tensor.matmul(
                            avp[:, hl, :],
                            pt[:, t2t * BLK + t1t * 128:t2t * BLK + t1t * 128 + 128],
                            V_all[:, 2 * b + t2t, h * 65:(h + 1) * 65],
                            start=(t2t == 0), stop=(t2t == 1),
                        )))
                return out

            def oproj_mms(b):
                out = []
                for mtl in range(2):
                    for on in range(2):
                        st = {}
                        for kt in range(DT):
                            def mm(mtl=mtl, on=on, kt=kt, st=st):
                                if kt == 0 and on == 0:
                                    y_t[mtl % 2] = y_pool.tile(
                                        [128, Q_DIM], bf16, tag="y", name="y"
                                    )
                                if kt == 0:
                                    st["ps"] = ps_proj.tile(
                                        [128, 512], f32, tag="proj", name="yps"
                                    )
                                ms = (2 * b + mtl) * 128
                                nc.tensor.matmul(
                                    st["ps"],
                                    OT_sb[:, kt, ms:ms + 128],
                                    wo_sb[:, kt, on * 512:(on + 1) * 512],
                                    start=(kt == 0), stop=(kt == DT - 1),
                                )
                                if kt == DT - 1:
                                    nc.vector.tensor_add(
                                        y_t[mtl % 2][:, on * 512:(on + 1) * 512],
                                        st["ps"],
                                        bo_sb[:, on * 512:(on + 1) * 512],
                                    )
                                    if on == 1:
                                        ms = (2 * b + mtl) * 128
                                        nc.gpsimd.dma_start(
                                            out=y[ms:ms + 128, :], in_=y_t[mtl % 2]
                                        )
                            out.append(mm)
                return out

            for slot in range(NB + 2):
                sb = slot if slot < NB else None
                ab = slot - 1 if 1 <= slot <= NB else None
                if slot == 0:
                    bigs = q_proj_mms(1, range(0, 4))
                elif slot == 1:
                    bigs = q_proj_mms(1, range(4, 8))
                else:
                    bigs = oproj_mms(slot - 2)

                if sb is None and ab is None:
                    for tr in tr_pend:
                        tr()
                    tr_pend.clear()
                    for g in bigs:
                        g()
                    continue

                big_iter = iter(bigs)

                for hg in range(8):
                    for tr in tr_pend[:2]:
                        tr()
                    del tr_pend[:2]
                    s_list = []
                    if sb is not None:
                        sts = [
                            ps_st.tile([128, 512], f32, tag="stps", name=f"st{i}")
                            for i in range(2)
                        ]
                        s_list = score_mms(sb, hg, sts)
                    a_list = []
                    if ab is not None:
                        avp = ps_av.tile([128, 4, 65], f32, tag="avps", name="avp")
                        a_list = av_mms(ab, hg, avp)

                    # 1:1 big:small — every AV matmul follows a stream-rich
                    # matmul so its stationary load is hidden.
                    for u in range(4):
                        if s_list:
                            s_list[u]()
                        if a_list:
                            a_list[2 * u]()
                        g = next(big_iter, None)
                        if g is not None:
                            g()
                        if a_list:
                            a_list[2 * u + 1]()

                    if sb is not None:
                        for hh in range(2):
                            pt = pt_pool.tile([128, 512], bf16, tag="pt", name="pt")
                            nc.scalar.activation(pt, sts[hh], Exp, scale=SCALE)
                            pts[(sb, hg, hh)] = pt

                    if ab is not None:
                        t1t, i = hg % 2, hg // 2
                        if i % 2 == 0:
                            osb_t[(ab, t1t, i // 2)] = osb_pool.tile(
                                [128, 512], bf16, tag="osb", name=f"osb{t1t}"
                            )
                        rc = rc_pool.tile([128, 4], f32, tag="rc", name="rc")
                        nc.vector.reciprocal(
                            out=rc,
                            in_=avp[:, :, 64:65].rearrange("p h c -> p (h c)"),
                        )
                        nc.vector.tensor_mul(
                            osb_t[(ab, t1t, i // 2)].rearrange(
                                "p (h c) -> p h c", c=64
                            )[:, 4 * (i % 2):4 * (i % 2) + 4, :],
                            avp[:, :, 0:64],
                            rc.rearrange("p (h o) -> p h o", o=1).to_broadcast(
                                [128, 4, 64]
                            ),
                        )
                        piece = osb_t[(ab, t1t, i // 2)]
                        for jj in range(2):
                            def tr(jt=2 * i + jj, piece=piece, t1t=t1t,
                                   ts=ab * BLK, eng=(i + jj) % 2):
                                trp = ps_tr.tile(
                                    [128, 128], bf16, tag="trps", name="trp"
                                )
                                nc.tensor.transpose(
                                    trp,
                                    piece[:, (jt % 4) * 128:(jt % 4 + 1) * 128],
                                    ident,
                                )
                                dst = OT_sb[:, jt, ts + t1t * 128:
                                            ts + (t1t + 1) * 128]
                                if eng == 0:
                                    nc.vector.tensor_copy(dst, trp)
                                else:
                                    nc.scalar.copy(dst, trp)
                            tr_pend.append(tr)

                # leftover bigs of this slot (shouldn't happen, but flush)
                for g in big_iter:
                    g()

    _split_multi_waits(nc, mybir)
    return nc


def _expected_mask():
    fid = np.repeat(np.arange(NUM_FRAMES), NUM_PATCHES)
    return (fid[:, None] == fid[None, :])[None, None]


def _reference_fallback(x, context, ln_gamma, ln_beta, Wq, Wkv, Wo, bo, mask):
    """Pure-numpy fallback for a non-block-diagonal mask (correctness only)."""
    x64 = x.astype(np.float64)
    mu = x64.mean(-1, keepdims=True)
    var = ((x64 - mu) ** 2).mean(-1, keepdims=True)
    xn = (x64 - mu) / np.sqrt(var + LN_EPS) * ln_gamma + ln_beta
    q = xn @ Wq.astype(np.float64)
    kv = context.astype(np.float64) @ Wkv.astype(np.float64)
    k, v = kv[..., :INNER], kv[..., INNER:]
    sh = lambda t: t.reshape(B, T, HEADS, DIM_HEAD).transpose(0, 2, 1, 3)
    q, k, v = sh(q), sh(k), sh(v)
    dots = np.einsum("bhnd,bhmd->bhnm", q, k) * SCALE
    dots = np.where(mask, dots, -np.inf)
    dots -= dots.max(-1, keepdims=True)
    e = np.exp(dots)
    attn = e / e.sum(-1, keepdims=True)
    out = np.einsum("bhnm,bhmd->bhnd", attn, v)
    out = out.transpose(0, 2, 1, 3).reshape(B, T, INNER)
    return (out @ Wo.astype(np.float64) + bo).astype(np.float32)


def _tile128(a):
    """[R, C] -> [128, (R/128)*C] partition-major pre-tiling for one-shot
    contiguous DMA into an SBUF [128, R/128, C] tile."""
    r, c = a.shape
    return np.ascontiguousarray(
        a.reshape(r // 128, 128, c).transpose(1, 0, 2).reshape(128, -1)
    )


def _prep_in_maps(x, context, ln_gamma, ln_beta, Wq, Wkv, Wo, bo):
    import ml_dtypes

    bf = ml_dtypes.bfloat16
    wq_eff = (ln_gamma[:, None] * Wq).astype(np.float32)
    wsum_neg = (-wq_eff.sum(axis=0, dtype=np.float64)).astype(np.float32)[None, :]
    bias_q = (ln_beta @ Wq).astype(np.float32)[None, :]
    wq_t = _tile128(wq_eff.astype(bf))
    wk_t = _tile128(np.ascontiguousarray(Wkv[:, :INNER]).astype(bf))
    wv_t = _tile128(np.ascontiguousarray(Wkv[:, INNER:]).astype(bf))
    wo_t = _tile128(Wo.astype(bf))
    bo2 = bo.astype(np.float32)[None, :]
    ones128 = np.ones((1, 128), np.float32)

    x_flat = x.reshape(B * T, Q_DIM)
    c_flat = context.reshape(B * T, KV_DIM)
    in_maps = []
    for c in range(N_CORES):
        sl = slice(c * TOK, (c + 1) * TOK)
        xT_t = _tile128(np.ascontiguousarray(x_flat[sl].T.astype(bf)))
        ctxT_t = _tile128(np.ascontiguousarray(c_flat[sl].T.astype(bf)))
        in_maps.append({
            "xT": xT_t, "ctxT": ctxT_t,
            "wq": wq_t, "wk": wk_t, "wv": wv_t, "wo": wo_t,
            "wsum_neg": wsum_neg, "bias_q": bias_q, "bo": bo2,
            "ones_in": ones128,
        })
    return in_maps


def _run(inputs, trace=False):
    from concourse.bass_utils import run_bass_kernel_spmd

    has_beta = bool(np.any(np.asarray(inputs["ln_beta"])))
    key = ("nc", has_beta)
    if key not in _CACHE:
        _CACHE[key] = _build_nc(has_beta)
    nc = _CACHE[key]
    in_maps = _prep_in_maps(
        inputs["x"], inputs["context"], inputs["ln_gamma"], inputs["ln_beta"],
        inputs["Wq"], inputs["Wkv"], inputs["Wo"], inputs["bo"],
    )
    res = run_bass_kernel_spmd(nc, in_maps, list(range(N_CORES)), trace=trace)
    ys = [np.asarray(res.results[c]["y"]).astype(np.float32)
          for c in range(N_CORES)]
    out = np.concatenate(ys, axis=0)
    return out.reshape(B, T, Q_DIM), res


def kernel(x, context, ln_gamma, ln_beta, Wq, Wkv, Wo, bo, mask):
    mask = np.asarray(mask)
    if not np.array_equal(mask, _expected_mask()):
        return _reference_fallback(
            np.asarray(x), np.asarray(context), np.asarray(ln_gamma),
            np.asarray(ln_beta), np.asarray(Wq), np.asarray(Wkv),
            np.asarray(Wo), np.asarray(bo), mask,
        )
    inputs = dict(x=np.asarray(x), context=np.asarray(context),
                  ln_gamma=np.asarray(ln_gamma), ln_beta=np.asarray(ln_beta),
                  Wq=np.asarray(Wq), Wkv=np.asarray(Wkv), Wo=np.asarray(Wo),
                  bo=np.asarray(bo))
    out, _ = _run(inputs, trace=False)
    return out


def _install_profiling_shims():
    """Enable the NTFF profile path under axon in this trimmed container:
    provide the antenv.axon_hooks registry and stub the artifact upload."""
    import sys
    import types

    if "antenv.axon_hooks" not in sys.modules:
        import antenv

        mod = types.ModuleType("antenv.axon_hooks")
        mod._hook = None

        def set_axon_ntff_profile_hook(h):
            mod._hook = h

        def get_axon_ntff_profile_hook():
            return mod._hook

        mod.set_axon_ntff_profile_hook = set_axon_ntff_profile_hook
        mod.get_axon_ntff_profile_hook = get_axon_ntff_profile_hook
        sys.modules["antenv.axon_hooks"] = mod
        antenv.axon_hooks = mod

    mod = sys.modules["antenv.axon_hooks"]
    if mod._hook is None:
        from trn_agent_boot.trn_boot import _ntff_profile_via_ctypes

        mod.set_axon_ntff_profile_hook(
            _ntff_profile_via_ctypes("/opt/axon/libaxon_pjrt.so")
        )

    from concourse import bass_utils

    if not getattr(bass_utils, "_upload_stubbed", False):
        bass_utils.upload_artifacts = lambda tmpdir: tmpdir
        bass_utils._upload_stubbed = True


def kernel_traced(**inputs):
    """Like kernel() but returns (out, BassKernelResults) with profiling."""
    _install_profiling_shims()
    out, res = _run(inputs, trace=True)
    return out, res
